# revision 9
# baseline (speedup 1.0000x reference)
"""CURLoRA layer kernel for 8 TRN2 NeuronCores.

Computes out = x @ (W + C@U@R)^T + bias for
  x: (4, 2048, 4096) f32, W: (4096, 4096), C: (4096, 64), U: (64, 64),
  R: (64, 4096), bias: (4096,)  ->  out: (4, 2048, 4096) f32

Sharding: 8 cores = 2 token-groups x 4 output-column-groups.
Each core computes out[tg, og] = x[tg] @ (W[og] + C[og]@U@R)^T + bias[og]
independently (no collectives needed).

Per-core kernel (bf16 compute, fp32 accumulate). Resource plan, driven
by three measured constraints: the PE matmul roofline (~470us), the
SWDGE cast-DMA stream (only ~80 GB/s write-side), and HWDGE ring FIFO
semantics (an instruction whose deps aren't ready parks its whole ring):
  1. W'^T [128d, 32k, 1024o] bf16: W f32 half-rows on the Sync HWDGE
     ring, adapter C@U@R added on DVE, PE-transposed (fast, during the
     prologue when the PE is otherwise idle).
  2. x tiles 0-2: SWDGE cast-DMA + PE transposes (SWDGE is otherwise
     idle; covers the W-build window).
  3. x tiles 3..31: f32 half-tiles on the Sync ring, cast to bf16 by
     Scalar-engine copies, then transposed to [128d, 32k, 128t] by the
     DMA xbar issued FROM THE SCALAR QUEUE: FIFO order guarantees the
     casts are complete when the xbar issues, so it never parks, and
     the PE does zero transpose work in steady state.
  4. out: PSUM accumulate over 32 k-tiles, DVE eviction + bias add,
     f32 stores on the Sync ring (its instructions never park: loads
     depend only on buffer recycling, stores on completed evictions).
"""

import sys

if "/opt/trn_rl_repo" not in sys.path:
    sys.path.insert(0, "/opt/trn_rl_repo")

import numpy as np

B, S, D = 4, 2048, 4096
O = 4096
RK = 64
T = B * S  # 8192 tokens
NT, NO = 2, 4  # token groups x out-column groups
TSH = T // NT  # 4096 tokens per core
OSH = O // NO  # 1024 out columns per core
N_CORES = 8

_CACHE = {}


def _build():
    from concourse import bacc
    import concourse.bass as bass
    import concourse.mybir as mybir
    from concourse.bass import ts
    from concourse.tile import TileContext
    from concourse.masks import make_identity

    f32 = mybir.dt.float32
    bf16 = mybir.dt.bfloat16

    nc = bacc.Bacc()
    x_ext = nc.declare_dram_parameter("x", [TSH, D], f32, isOutput=False)
    w_ext = nc.declare_dram_parameter("W", [OSH, D], f32, isOutput=False)
    c_ext = nc.declare_dram_parameter("C", [OSH, RK], f32, isOutput=False)
    u_ext = nc.declare_dram_parameter("U", [RK, RK], f32, isOutput=False)
    r_ext = nc.declare_dram_parameter("R", [RK, D], f32, isOutput=False)
    b_ext = nc.declare_dram_parameter("bias", [OSH], f32, isOutput=False)
    out_ext = nc.declare_dram_parameter("out", [TSH, OSH], f32, isOutput=True)

    NKT = D // 128  # 32 k-tiles
    NTT = TSH // 128  # 32 t-tiles per core
    NOJ = OSH // 512  # 2 o-blocks of 512
    NWJ = OSH // 128  # 8 W row-tiles
    HD = D // 2  # half tile for f32 staging
    AHEAD = 2  # x tiles requested ahead of their matmuls
    NSW = 3  # leading x tiles on the SWDGE cast + PE transpose path

    with TileContext(nc) as tc:
        with (
            tc.tile_pool(name="const", bufs=1) as const,
            tc.tile_pool(name="wt", bufs=1) as wtp,
            tc.tile_pool(name="small", bufs=1) as small,
            tc.tile_pool(name="f32st", bufs=3) as f32st,
            tc.tile_pool(name="w2p", bufs=2) as w2p,
            tc.tile_pool(name="xpool", bufs=2) as xpool,
            tc.tile_pool(name="xtpool", bufs=AHEAD + 2) as xtpool,
            tc.tile_pool(name="opool", bufs=2) as opool,
            # PSUM: psT (t 3x1 + s 1x1) + psA 4x1 = 8 banks
            tc.tile_pool(name="psT", bufs=3, space="PSUM") as psT,
            tc.tile_pool(name="psA", bufs=4, space="PSUM") as psA,
        ):
            ident = const.tile([128, 128], bf16)
            make_identity(nc, ident)

            # resident W'^T: [128 d-part, 32 k-tiles, 1024 o] bf16
            wt_sb = wtp.tile([128, NKT, OSH], bf16)
            bias_sb = const.tile([128, OSH], f32)

            # small adapter inputs first on the SWDGE queue (they gate the
            # adapter chain)
            u_sb = small.tile([RK, RK], bf16)
            nc.gpsimd.dma_start(out=u_sb[:], in_=u_ext[:])  # cast f32->bf16
            c_nat = small.tile([128, OSH // 128, RK], bf16)
            nc.gpsimd.dma_start(
                out=c_nat[:], in_=c_ext[:].rearrange("(j p) r -> p j r", p=128)
            )
            r_sb = small.tile([RK, D], bf16)
            nc.gpsimd.dma_start(out=r_sb[:], in_=r_ext[:])
            ut_sb = small.tile([RK, RK], bf16)
            ct_sb = small.tile([RK, OSH], bf16)
            ur_sb = small.tile([RK, D], bf16)

            # bias broadcast to all 128 partitions
            b_ap = b_ext[:]
            b_bc = bass.AP(
                tensor=b_ap.tensor,
                offset=b_ap.offset,
                ap=[[0, 128]] + [list(p) for p in b_ap.ap],
            )
            nc.gpsimd.dma_start(out=bias_sb[:], in_=b_bc)

            # U^T (tiny PE transpose)
            ps_u = psT.tile([RK, 128], bf16, tag="s", bufs=1)
            nc.tensor.transpose(ps_u[:, :RK], u_sb[:], ident[:RK, :RK])
            nc.vector.tensor_copy(out=ut_sb[:], in_=ps_u[:, :RK])

            # C^T tiles (8 tiny PE transposes)
            for j in range(OSH // 128):
                ps_c = psT.tile([RK, 128], bf16, tag="s", bufs=1)
                nc.tensor.transpose(ps_c[:], c_nat[:, j, :], ident[:])
                nc.vector.tensor_copy(out=ct_sb[:, ts(j, 128)], in_=ps_c[:])

            # UR = U @ R  -> [64, 4096] bf16
            for c in range(D // 512):
                ps_ur = psA.tile([128, 512], f32, tag="a")
                nc.tensor.matmul(
                    ps_ur[:RK, :], ut_sb[:], r_sb[:, ts(c, 512)], start=True, stop=True
                )
                nc.vector.tensor_copy(out=ur_sb[:, ts(c, 512)], in_=ps_ur[:RK, :])

            def w_build_tile(j):
                # W' row-tile j: f32 half-rows via the Sync ring, adapter
                # added on DVE into bf16 w2, PE-transposed into W'^T.
                wh = [f32st.tile([128, HD], f32, name="wh") for _ in range(2)]
                for h in range(2):
                    nc.sync.dma_start(wh[h][:], w_ext[ts(j, 128), ts(h, HD)])
                w2 = w2p.tile([128, D], bf16, name="w2")
                for c in range(D // 512):
                    ps_ad = psA.tile([128, 512], f32, tag="a")
                    nc.tensor.matmul(
                        ps_ad[:],
                        ct_sb[:, ts(j, 128)],
                        ur_sb[:, ts(c, 512)],
                        start=True,
                        stop=True,
                    )
                    h, ch = divmod(c, D // 1024)
                    nc.vector.tensor_add(
                        out=w2[:, ts(c, 512)],
                        in0=ps_ad[:],
                        in1=wh[h][:, ts(ch, 512)],
                    )
                for g in range(NKT // 8):
                    ps_wt = psT.tile([128, 8, 128], bf16, tag="t", bufs=3)
                    for ii in range(8):
                        nc.tensor.transpose(
                            ps_wt[:, ii, :], w2[:, ts(8 * g + ii, 128)], ident[:]
                        )
                    nc.vector.tensor_copy(
                        out=wt_sb[:, 8 * g : 8 * g + 8, ts(j, 128)], in_=ps_wt[:]
                    )

            def emit_x_acquire(i):
                xT = xtpool.tile([128, NKT, 128], bf16)
                x_nat = xpool.tile([128, D], bf16)
                if i < NSW:
                    # SWDGE cast-DMA + PE transposes (prologue path)
                    nc.gpsimd.dma_start(out=x_nat[:], in_=x_ext[ts(i, 128), :])
                    for g in range(NKT // 8):
                        ps_xt = psT.tile([128, 8, 128], bf16, tag="t", bufs=3)
                        for ii in range(8):
                            nc.tensor.transpose(
                                ps_xt[:, ii, :], x_nat[:, ts(8 * g + ii, 128)], ident[:]
                            )
                        nc.vector.tensor_copy(
                            out=xT[:, 8 * g : 8 * g + 8, :], in_=ps_xt[:]
                        )
                else:
                    # f32 halves on the Sync ring, Scalar-engine cast, then
                    # DMA xbar issued from the Scalar queue (deps satisfied
                    # by FIFO order -- never parks the ring).
                    for h in range(2):
                        xh = f32st.tile([128, HD], f32, name="xh")
                        nc.sync.dma_start(xh[:], x_ext[ts(i, 128), ts(h, HD)])
                        nc.scalar.copy(out=x_nat[:, ts(h, HD)], in_=xh[:])
                    nc.scalar.dma_start(out=xT[:], in_=x_nat[:], transpose=True)
                return xT

            def emit_mm_j(i, j, xT, out_sb):
                psm = psA.tile([128, 512], f32, tag="a")
                for k in range(NKT):
                    nc.tensor.matmul(
                        psm[:],
                        xT[:, k, :],
                        wt_sb[:, k, ts(j, 512)],
                        start=(k == 0),
                        stop=(k == NKT - 1),
                    )
                nc.vector.tensor_add(
                    out=out_sb[:, ts(j, 512)],
                    in0=psm[:],
                    in1=bias_sb[:, ts(j, 512)],
                )

            def emit_x_mm(i, xT):
                out_sb = opool.tile([128, OSH], f32, tag="out_sb")
                for j in range(NOJ):
                    emit_mm_j(i, j, xT, out_sb)
                    nc.sync.dma_start(
                        out_ext[ts(i, 128), ts(j, 512)], out_sb[:, ts(j, 512)]
                    )

            # W build with early x tiles interleaved; the PE fills the
            # W-DMA window with adapter mms + W'/early-x transposes.
            pre = {}
            for j in range(4):
                w_build_tile(j)
            pre[0] = emit_x_acquire(0)
            for j in range(4, NWJ):
                w_build_tile(j)
            pre[1] = emit_x_acquire(1)
            pre[2] = emit_x_acquire(2)
            next_req = 3

            # ---------------- main loop: stream x ----------------
            for i in range(NTT):
                while next_req <= min(NTT - 1, i + AHEAD):
                    pre[next_req] = emit_x_acquire(next_req)
                    next_req += 1
                emit_x_mm(i, pre.pop(i))

    nc.compile()
    return nc


def kernel(x, W, C, U, R, bias):
    from concourse.bass_utils import run_bass_kernel_spmd

    x = np.ascontiguousarray(np.asarray(x, dtype=np.float32)).reshape(T, D)
    W = np.ascontiguousarray(np.asarray(W, dtype=np.float32))
    C = np.ascontiguousarray(np.asarray(C, dtype=np.float32))
    U = np.ascontiguousarray(np.asarray(U, dtype=np.float32))
    R = np.ascontiguousarray(np.asarray(R, dtype=np.float32))
    bias = np.ascontiguousarray(np.asarray(bias, dtype=np.float32))

    if "nc" not in _CACHE:
        _CACHE["nc"] = _build()
    nc = _CACHE["nc"]

    in_maps = []
    for core in range(N_CORES):
        tg, og = divmod(core, NO)
        in_maps.append(
            {
                "x": x[tg * TSH : (tg + 1) * TSH],
                "W": W[og * OSH : (og + 1) * OSH],
                "C": C[og * OSH : (og + 1) * OSH],
                "U": U,
                "R": R,
                "bias": bias[og * OSH : (og + 1) * OSH],
            }
        )

    res = run_bass_kernel_spmd(nc, in_maps, core_ids=list(range(N_CORES)))

    out = np.empty((T, O), dtype=np.float32)
    for core in range(N_CORES):
        tg, og = divmod(core, NO)
        out[tg * TSH : (tg + 1) * TSH, og * OSH : (og + 1) * OSH] = res.results[core][
            "out"
        ]
    return out.reshape(B, S, O)
